# revision 8
# baseline (speedup 1.0000x reference)
"""GPT-NeoX attention layer as a Bass/Tile kernel for 8 Trainium2 NeuronCores.

Problem: hidden[2048,1,4096] -> QKV proj (W[4096,12288]) -> 32-head attention
(head_dim 128, rotary on first 32 dims, causal) -> dense proj (W[4096,4096]).

Sharding: tensor-parallel over heads (4 heads/core). The host replicates
hidden^T (fp16) to every core, so there is no hidden AllGather and no
on-chip transpose phase at all. Each core:
  P1: QKV projection for its 4 heads, one 512-seq block at a time. q/k are
      produced TRANSPOSED ([head_dim, seq], via a host-side column
      permutation of W_qkv so the rotary dims of the 4 heads stack into full
      128-partition tiles); v produced in [seq, head_dim] layout. Rotary
      applied on-chip with host cos/sin tables (rotate_half via a
      partition-permuting SBUF->SBUF DMA, sign baked into the sin table).
  P2: attention for i-block sb runs IMMEDIATELY after P1(sb) (causality:
      k/v for j<=sb are ready), so each i-block's ctx AllGather fires early
      and hides under later compute: scores^T tiles [kv 128 x q 512] on PE
      (contraction = head_dim), additive causal mask on diagonal tiles, exp
      on ScalarE (no max-subtraction: scores are O(10)), denominator via
      ones-matmul partition reduction, PV matmul accumulates ctx^T,
      normalization by 1/denom broadcast through a rank-1 matmul.
  P3: ctx AllGathers per i-block, interleaved as above; the last one hides
      under the dense projection of earlier i-blocks.
  P4: dense projection, column-sharded: out[:, c*512:(c+1)*512].
Host gathers by concatenating the 8 column slices.

dtypes: large streamed tensors (weights, hidden^T, ctx) are fp16 to halve
DMA traffic (PE rate is identical); attention math (q/k/v tiles, exp,
denominators) is f32r; softmax/psum accumulation fp32. Collectives get the
gpsimd queue to themselves; other small DMAs ride scalar/vector queues.
"""
import sys
import os

sys.path.insert(0, "/opt/trn_rl_repo")

import numpy as np

import concourse.bacc as bacc
import concourse.mybir as mybir
import concourse.tile as tile

SEQ = 2048
HIDDEN = 4096
HEADS = 32
HD = 128
ROT = 32
HALF = ROT // 2  # 16
N_CORES = 8
HPC = HEADS // N_CORES       # 4 heads per core
CW = HPC * HD                # 512 columns of work per core (v / ctx / dense)
KT = HIDDEN // 128           # 32 k-tiles over the hidden dim
SB = 512                     # sequence block for QKV + attention i-blocks
NSB = SEQ // SB              # 4
NST = SEQ // 128             # 16 sequence tiles
NEG = -1.0e9                 # additive mask value (pre-scale)
SCALE = float(1.0 / np.sqrt(HD))

F32 = mybir.dt.float32
F32R = mybir.dt.float32r
F16 = mybir.dt.float16
AF = mybir.ActivationFunctionType

_CACHE = {}
_ZBIAS = [False]


def _f32(ap):
    return ap.bitcast(F32)


def _build_program(rep=1, trace_sim=False, skip_cc=False, phases="all"):
    nc = bacc.Bacc("TRN2", target_bir_lowering=False, debug=False,
                   num_devices=N_CORES)

    # ---- I/O ---------------------------------------------------------------
    # hidden^T, replicated to every core by the host (fp16)
    hid_t = nc.dram_tensor("hid_t", [HIDDEN, SEQ], F16, kind="ExternalInput")
    # w_qk: [m_tile, k_tile, 128, 128] fp16, column-permuted (see _host_prep)
    w_qk = nc.dram_tensor("w_qk", [8, KT, 128, 128], F16, kind="ExternalInput")
    w_v = nc.dram_tensor("w_v", [KT, 128, CW], F16, kind="ExternalInput")
    w_d = nc.dram_tensor("w_d", [KT, 128, CW], F16, kind="ExternalInput")
    b_qk = nc.dram_tensor("b_qk", [128, 8], F32, kind="ExternalInput")
    b_v = nc.dram_tensor("b_v", [1, CW], F16, kind="ExternalInput")
    b_d = nc.dram_tensor("b_d", [1, CW], F16, kind="ExternalInput")
    cos_in = nc.dram_tensor("cos_in", [128, SEQ], F16, kind="ExternalInput")
    sin_in = nc.dram_tensor("sin_in", [128, SEQ], F16, kind="ExternalInput")
    mask_in = nc.dram_tensor("mask_in", [128, 4 * SB],
                             mybir.dt.bfloat16, kind="ExternalInput")
    ones_col_in = nc.dram_tensor("ones_col_in", [128, 1], F32R,
                                 kind="ExternalInput")
    ones_row_in = nc.dram_tensor("ones_row_in", [1, 128], F32R,
                                 kind="ExternalInput")
    ones_row16_in = nc.dram_tensor("ones_row16_in", [1, 128], F16,
                                   kind="ExternalInput")
    out = nc.dram_tensor("out", [SEQ, CW], F32, kind="ExternalOutput")

    rg = [list(range(N_CORES))]

    with tile.TileContext(nc, trace_sim=trace_sim) as tc:
        with (
            tc.tile_pool(name="const", bufs=1) as constp,
            tc.tile_pool(name="dram", bufs=1, space="DRAM") as dramp,
        ):
            # constants
            ones_col = constp.tile([128, 1], F32R)
            negone = constp.tile([128, 1], F32)
            ones_row = constp.tile([1, 128], F32R)
            ones_row16 = constp.tile([1, 128], F16)
            bqk_sb = constp.tile([128, 8], F32)
            bv_sb = constp.tile([1, CW], F16)
            bd_sb = constp.tile([1, CW], F16)
            cos_sb = constp.tile([128, SEQ], F16)
            sin_sb = constp.tile([128, SEQ], F16)
            mask_sb = constp.tile([128, 4 * SB], mybir.dt.bfloat16)
            nc.sync.dma_start(ones_col[:], ones_col_in[:])
            nc.any.memset(negone, -1.0)
            nc.sync.dma_start(ones_row[:], ones_row_in[:])
            nc.sync.dma_start(ones_row16[:], ones_row16_in[:])
            nc.sync.dma_start(bqk_sb[:], b_qk[:])
            nc.sync.dma_start(bv_sb[:], b_v[:])
            nc.sync.dma_start(bd_sb[:], b_d[:])
            nc.sync.dma_start(cos_sb[:], cos_in[:])
            nc.sync.dma_start(sin_sb[:], sin_in[:])
            nc.sync.dma_start(mask_sb[:], mask_in[:])

            for _rep in range(rep):
              # ctx collective bounce buffers, one per i-block (fresh per rep)
              # i-blocks 0 and 1 share one ccin/ccout pair so their ctx
              # rides a single AllGather launch (collective launches are
              # expensive here); blocks 2 and 3 stay separate because their
              # dense consumers gate on them individually at the tail.
              ccin01 = dramp.tile([CW, 2 * SB], F16,
                                  name=f"ccin01_{_rep}")
              ccout01 = dramp.tile([HIDDEN, 2 * SB], F16,
                                   addr_space="Shared",
                                   name=f"ccout01_{_rep}")
              ccin_ctx = [None, None] + [
                  dramp.tile([CW, SB], F16, name=f"ccin_ctx{_rep}_{i}")
                  for i in (2, 3)]
              ccout_ctx = [None, None] + [
                  dramp.tile([HIDDEN, SB], F16, addr_space="Shared",
                             name=f"ccout_ctx{_rep}_{i}")
                  for i in (2, 3)]

              # persistent QKV outputs (live through P1+P2)
              with tc.tile_pool(name="qkvout", bufs=1) as qkvp:
                  qh = [qkvp.tile([128, SEQ], F16, name=f"qh{h}")
                        for h in range(HPC)]
                  kh = [qkvp.tile([128, SEQ], F16, name=f"kh{h}")
                        for h in range(HPC)]
                  vsb = [qkvp.tile([128, CW], F16, name=f"v{s}")
                         for s in range(NST)]

                  # W_dense prefetch pool spans P1+P2+P4; tiles are
                  # loaded in slices between s-blocks (never ahead of the
                  # startup-critical hidden loads on the sync queue)
                  wdp_ctx = tc.tile_pool(name="wdp", bufs=1)
                  wdp = wdp_ctx.__enter__()
                  wd_sb = []

                  def prefetch_wd(k0, k1):
                      if phases != "all":
                          return
                      for k in range(k0, k1):
                          w_t = wdp.tile([128, CW], F16, name=f"wd{k}")
                          nc.scalar.dma_start(w_t[:], w_d[k].opt())
                          wd_sb.append(w_t)

                  with (
                      tc.tile_pool(name="htp", bufs=10) as htp,
                      tc.tile_pool(name="wqp", bufs=4) as wqp,
                      tc.tile_pool(name="wvp", bufs=2) as wvp,
                      tc.tile_pool(name="rotp", bufs=2) as rotp,
                      tc.tile_pool(name="rscp", bufs=6) as rscp,
                      tc.tile_pool(name="exp", bufs=6) as exp_p,
                      tc.tile_pool(name="accp", bufs=3) as accp,
                      tc.tile_pool(name="rcp", bufs=3) as rcp,
                      tc.tile_pool(name="rbp", bufs=3) as rbp,
                      tc.tile_pool(name="ctxp", bufs=3) as ctxp,
                  ):
                      def rope(rot_t, dst, sb):
                          """rot_t: [128, SB], rows hl*32+d = rotary dim d of
                          head hl. rotate_half is materialized by a
                          partition-permuting SBUF->SBUF DMA; the sign lives
                          in the sin table."""
                          cs = cos_sb[:, sb * SB:(sb + 1) * SB]
                          sn = sin_sb[:, sb * SB:(sb + 1) * SB]
                          shf = rscp.tile([128, SB], F32R, name="rsc")
                          for hl in range(HPC):
                              r = hl * ROT
                              nc.scalar.dma_start(shf[r:r + HALF, :],
                                                  rot_t[r + HALF:r + ROT, :])
                              nc.scalar.dma_start(shf[r + HALF:r + ROT, :],
                                                  rot_t[r:r + HALF, :])
                          t1 = rscp.tile([128, SB], F32R, name="rsc")
                          t2 = rscp.tile([128, SB], F32R, name="rsc")
                          rp = rscp.tile([128, SB], F32R, name="rsc")
                          nc.vector.tensor_mul(t1[:], _f32(rot_t[:]), cs)
                          nc.vector.tensor_mul(t2[:], _f32(shf[:]), sn)
                          nc.vector.tensor_add(rp[:], _f32(t1[:]), _f32(t2[:]))
                          for hl in range(HPC):
                              nc.scalar.activation(
                                  dst[hl][0:ROT, sb * SB:(sb + 1) * SB],
                                  rp[hl * ROT:(hl + 1) * ROT, :], AF.Copy)

                      def evac_qk(m, pq, sb):
                          scols = slice(sb * SB, (sb + 1) * SB)
                          if m == 0 or m == 1:
                              rot_t = rotp.tile([128, SB], F32R, name="rot_t")
                              nc.scalar.activation(rot_t[:], pq[:], AF.Identity,
                                                   bias=bqk_sb[:, m:m + 1])
                              rope(rot_t, qh if m == 0 else kh, sb)
                          else:
                              # 32-row chunks: compute-engine partition accesses
                              # >32 rows must start at partition 0; head spans
                              # (96 rows) are exactly 3 chunks.
                              t = (m - 2) % 3
                              dst = qh if m <= 4 else kh
                              for ch in range(4):
                                  g = t * 128 + ch * 32
                                  hl = g // 96
                                  dlo = 32 + g - hl * 96
                                  nc.scalar.activation(
                                      dst[hl][dlo:dlo + 32, scols],
                                      pq[ch * 32:(ch + 1) * 32, :], AF.Identity,
                                      bias=bqk_sb[ch * 32:(ch + 1) * 32,
                                                  m:m + 1])

                      def qkv_sb(sb):
                        with (
                            tc.tile_pool(name="qkps", bufs=2,
                                         space="PSUM") as qkps,
                            tc.tile_pool(name="vps", bufs=4,
                                         space="PSUM") as vps,
                        ):
                          # first weight strips load ahead of the hidden
                          # bulk so the first matmuls of the s-block are not
                          # queued behind 4MB of hidden DMA.
                          wvb0 = wvp.tile([128, 4 * CW], F16, name="wvb")
                          nc.sync.dma_start(
                              wvb0[:].rearrange("p (k c) -> p k c", k=4),
                              w_v[0:4].rearrange("k p c -> p k c"))
                          wqb0 = wqp.tile([128, 16 * 128], F16, name="wqb")
                          nc.sync.dma_start(
                              wqb0[:].rearrange("p (k c) -> p k c", k=16),
                              w_qk[0, 0:16].rearrange("k p c -> p k c"))
                          # hidden^T k-tiles for this s-block, straight from
                          # the replicated input: 8 tiles of [128, 4*SB] fp16
                          ht4 = []
                          for kg in range(8):
                              h4 = htp.tile([128, 4 * SB], F16, name="ht4")
                              nc.sync.dma_start(
                                  h4[:].rearrange("p (k s) -> p k s", k=4),
                                  hid_t[kg * 512:(kg + 1) * 512,
                                        sb * SB:(sb + 1) * SB].rearrange(
                                      "(k p) s -> p k s", k=4))
                              ht4.append(h4)

                          def htk(k):
                              return ht4[k // 4][:, (k % 4) * SB:
                                                 (k % 4 + 1) * SB]

                          def v_part():
                              # k-outer, 4 psum banks held over the k sweep
                              pv = [vps.tile([128, CW], F32, name="pv")
                                    for _ in range(4)]
                              wvb = wvb0
                              for k in range(KT):
                                  if k % 4 == 0 and k > 0:
                                      wvb = wvp.tile([128, 4 * CW], F16,
                                                     name="wvb")
                                      nc.sync.dma_start(
                                          wvb[:].rearrange(
                                              "p (k c) -> p k c", k=4),
                                          w_v[k:k + 4].rearrange(
                                              "k p c -> p k c"))
                                  hk = htk(k)
                                  for q4 in range(4):
                                      nc.tensor.matmul(
                                          pv[q4][:],
                                          hk[:, q4 * 128:(q4 + 1) * 128],
                                          wvb[:, (k % 4) * CW:(k % 4 + 1) * CW],
                                          start=(k == 0),
                                          stop=(_ZBIAS[0] and k == KT - 1))
                              for q4 in range(4):
                                  if not _ZBIAS[0]:
                                      nc.tensor.matmul(pv[q4][:],
                                                       ones_row16[:],
                                                       bv_sb[:], start=False,
                                                       stop=True)
                                  nc.scalar.activation(vsb[sb * 4 + q4][:],
                                                       pv[q4][:], AF.Copy)

                          def qk_part():
                              # m-outer, k-inner; W strips batched (16 k/DMA)
                              wqb = wqb0
                              for m in range(8):
                                  pq = qkps.tile([128, SB], F32, name="pq")
                                  for k in range(KT):
                                      if k % 16 == 0 and not (m == 0
                                                              and k == 0):
                                          wqb = wqp.tile([128, 16 * 128], F16,
                                                         name="wqb")
                                          nc.sync.dma_start(
                                              wqb[:].rearrange(
                                                  "p (k c) -> p k c", k=16),
                                              w_qk[m, k:k + 16].rearrange(
                                                  "k p c -> p k c"))
                                      nc.tensor.matmul(
                                          pq[:],
                                          wqb[:, (k % 16) * 128:
                                              (k % 16 + 1) * 128],
                                          htk(k), start=(k == 0),
                                          stop=(k == KT - 1))
                                  evac_qk(m, pq, sb)

                          v_part()
                          qk_part()

                      def attn_head_main(ib, h, cps, sps):
                          """Scores/exp/PV for one head, with the score
                          matmuls software-pipelined two j-pairs ahead of PV:
                          the PE's in-order queue would otherwise stall each
                          PV on ScalarE's exp (~1us/pair). Returns (cp, rc)
                          for the deferred normalize tail."""
                          njt = 4 * (ib + 1)
                          np_ = njt // 2
                          cp = cps.tile([128, SB], F32, name="cp")
                          acc = accp.tile([128, SB], F32R, name="acc")

                          def emit_scores(jp):
                              # two j-tiles share one [128, 2*SB] psum so exp
                              # and the denominator add run once per pair.
                              # The final (diagonal) pair only computes the
                              # upper column half; the lower half is fully
                              # masked, and stale psum there is driven to
                              # exp()=0 by the -1e9 mask add.
                              diag = (jp == np_ - 1)
                              c0 = SB // 2 if diag else 0
                              sp = sps.tile([128, 2 * SB], F32, name="sp")
                              for u in range(2):
                                  jt = 2 * jp + u
                                  nc.tensor.matmul(
                                      sp[:, u * SB + c0:(u + 1) * SB],
                                      kh[h][:, jt * 128:(jt + 1) * 128],
                                      qh[h][:, ib * SB + c0:(ib + 1) * SB],
                                      start=True, stop=True)
                              def half(ap, w):
                                  # the two j-tiles' live columns [c0:SB) as
                                  # one strided AP [128, 2, SB-c0]
                                  return ap.rearrange("p (u c) -> p u c",
                                                      u=2)[:, :, SB - w:]
                              w = SB - c0
                              if 2 * jp + 1 >= 4 * ib:
                                  t = 2 * jp - 4 * ib
                                  msk = mask_sb[:, t * SB:(t + 2) * SB]
                                  nc.vector.tensor_add(
                                      half(sp[:], w), half(sp[:], w),
                                      half(msk, w))
                              ex = exp_p.tile([128, 2 * SB], F16, name="ex")
                              # exp only the live columns: the masked half of
                              # the diagonal pair is exactly 0 after exp, so
                              # the acc adds and PV below skip it too (the
                              # stale ex region is never read).
                              nc.scalar.activation(half(ex[:], w),
                                                   half(sp[:], w), AF.Exp,
                                                   scale=SCALE,
                                                   bias=negone[:])
                              if jp == 0:
                                  nc.vector.tensor_add(
                                      acc[:], ex[:, 0:SB],
                                      ex[:, SB:2 * SB])
                              else:
                                  nc.vector.tensor_add(
                                      acc[:, c0:SB], _f32(acc[:, c0:SB]),
                                      ex[:, c0:SB])
                                  nc.vector.tensor_add(
                                      acc[:, c0:SB], _f32(acc[:, c0:SB]),
                                      ex[:, SB + c0:2 * SB])
                              return ex

                          exs = {0: emit_scores(0)}
                          if np_ > 1:
                              exs[1] = emit_scores(1)
                          for jp in range(np_):
                              if jp + 2 < np_:
                                  exs[jp + 2] = emit_scores(jp + 2)
                              ex = exs.pop(jp)
                              c0 = SB // 2 if jp == np_ - 1 else 0
                              for u in range(2):
                                  jt = 2 * jp + u
                                  nc.tensor.matmul(
                                      cp[:, c0:SB],
                                      vsb[jt][:, h * 128:(h + 1) * 128],
                                      ex[:, u * SB + c0:(u + 1) * SB],
                                      start=(jt == 0),
                                      stop=(jt == njt - 1))
                          # denominator: ones-matmul partition reduction into
                          # a bank borrowed from the sp series (row 0 only)
                          dn = sps.tile([128, 2 * SB], F32, name="sp")
                          nc.tensor.matmul(dn[0:1, 0:SB], ones_col[:], acc[:],
                                           start=True, stop=True)
                          rc = rcp.tile([1, SB], F32R, name="rc")
                          with nc.allow_low_precision(
                                  reason="f32r: 11-bit mantissa is plenty "
                                         "for the softmax denominator"):
                              nc.vector.reciprocal(rc[:], dn[0:1, 0:SB])
                          return cp, rc

                      def attn_head_tail(ib, h, cps_unused, sps, cp, rc):
                          """rb broadcast + normalize + ctx store. Emitted
                          AFTER the next head's main block so the rb matmul
                          (which waits on the DVE reciprocal) reaches the PE
                          long after rc is ready — no PE stall."""
                          rb = sps.tile([128, 2 * SB], F32, name="sp")
                          nc.tensor.matmul(rb[:, 0:SB], ones_row[:], rc[:],
                                           start=True, stop=True)
                          rbs = rbp.tile([128, SB], F32R, name="rbs")
                          nc.scalar.activation(rbs[:], rb[:, 0:SB], AF.Copy)
                          ctxn = ctxp.tile([128, SB], F16, name="ctxn")
                          nc.vector.tensor_mul(ctxn[:], cp[:], _f32(rbs[:]))
                          if ib < 2:
                              nc.scalar.dma_start(
                                  ccin01[h * 128:(h + 1) * 128,
                                         ib * SB:(ib + 1) * SB], ctxn[:])
                          else:
                              nc.scalar.dma_start(
                                  ccin_ctx[ib][h * 128:(h + 1) * 128, :],
                                  ctxn[:])

                      def attn_ib(ib):
                          with (
                              tc.tile_pool(name="sps", bufs=3,
                                           space="PSUM") as sps,
                              tc.tile_pool(name="cps", bufs=2,
                                           space="PSUM") as cps,
                          ):
                              pend = None
                              for h in range(HPC):
                                  cur = attn_head_main(ib, h, cps, sps)
                                  if pend is not None:
                                      attn_head_tail(ib, h - 1, cps, sps,
                                                     *pend)
                                  pend = cur
                              attn_head_tail(ib, HPC - 1, cps, sps, *pend)
                              if not skip_cc and phases == "all" and ib != 0:
                                  cci = (ccin01 if ib == 1
                                         else ccin_ctx[ib])
                                  cco = (ccout01 if ib == 1
                                         else ccout_ctx[ib])
                                  nc.gpsimd.collective_compute(
                                      "AllGather", mybir.AluOpType.bypass,
                                      replica_groups=rg,
                                      ins=[cci[:].opt()],
                                      outs=[cco[:].opt()])

                      WD_SLICES = [0, 0, 11, 22, KT]
                      for sb in range(NSB):
                          prefetch_wd(WD_SLICES[sb], WD_SLICES[sb + 1])
                          qkv_sb(sb)
                          if phases != "p01":
                              attn_ib(sb)

                  # ---- P4: dense projection (column shard) -----------------
                  if phases == "p012":
                      nc.sync.dma_start(out[0:CW, 0:SB // 2],
                                        ccin01[0:CW, 0:SB].bitcast(F32))
                  if phases == "all":
                   with (
                       tc.tile_pool(name="ctp", bufs=5) as ctp,
                       tc.tile_pool(name="outp", bufs=3) as outp,
                       tc.tile_pool(name="pdps", bufs=5, space="PSUM") as pdps,
                   ):
                       for mq in range(4):
                           pd = [pdps.tile([128, CW], F32, name="pd")
                                 for _ in range(4)]
                           for k in range(KT):
                               k4 = k % 4
                               if k4 == 0:
                                   ct4 = ctp.tile([128, 4 * SB], F16, name="ct4")
                                   if mq < 2:
                                       csrc = ccout01[k * 128:(k + 4) * 128,
                                                      mq * SB:(mq + 1) * SB]
                                   else:
                                       csrc = ccout_ctx[mq][
                                           k * 128:(k + 4) * 128, :]
                                   nc.sync.dma_start(
                                       ct4[:].rearrange("p (k s) -> p k s", k=4),
                                       csrc.rearrange("(k p) s -> p k s", k=4))
                               ct = ct4[:, k4 * SB:(k4 + 1) * SB]
                               for m4 in range(4):
                                   nc.tensor.matmul(
                                       pd[m4][:], ct[:, m4 * 128:(m4 + 1) * 128],
                                       wd_sb[k][:], start=(k == 0),
                                       stop=(_ZBIAS[0] and k == KT - 1))
                           for m4 in range(4):
                               if not _ZBIAS[0]:
                                   nc.tensor.matmul(pd[m4][:], ones_row16[:],
                                                    bd_sb[:], start=False,
                                                    stop=True)
                               ot = outp.tile([128, CW], F32, name="ot")
                               nc.scalar.activation(ot[:], pd[m4][:], AF.Copy)
                               st = mq * 4 + m4
                               nc.sync.dma_start(out[st * 128:(st + 1) * 128, :],
                                                 ot[:])

                  wdp_ctx.__exit__(None, None, None)

    nc.compile()
    return nc
def _get_exec(rep=1):
    if ("exec", rep) in _CACHE:
        return _CACHE[("exec", rep)]
    import jax
    from jax.sharding import Mesh, PartitionSpec
    from jax.experimental.shard_map import shard_map
    from concourse import bass2jax

    nc = _build_program(rep=rep)
    bass2jax.install_neuronx_cc_hook()

    partition_name = (nc.partition_id_tensor.name
                      if nc.partition_id_tensor else None)
    in_names = []
    out_names = []
    out_avals = []
    zero_shapes = []
    for alloc in nc.m.functions[0].allocations:
        if not isinstance(alloc, mybir.MemoryLocationSet):
            continue
        name = alloc.memorylocations[0].name
        if alloc.kind == "ExternalInput":
            if name != partition_name:
                in_names.append(name)
        elif alloc.kind == "ExternalOutput":
            np_dt = mybir.dt.np(alloc.dtype)
            out_names.append(name)
            out_avals.append(
                jax.core.ShapedArray(tuple(alloc.tensor_shape), np_dt))
            zero_shapes.append((tuple(alloc.tensor_shape), np_dt))

    n_params = len(in_names)
    n_outs = len(out_names)
    all_in_names = in_names + out_names
    if partition_name is not None:
        all_in_names = all_in_names + [partition_name]
    donate = tuple(range(n_params, n_params + n_outs))

    def _body(*args):
        operands = list(args)
        if partition_name is not None:
            operands.append(bass2jax.partition_id_tensor())
        outs = bass2jax._bass_exec_p.bind(
            *operands,
            out_avals=tuple(out_avals),
            in_names=tuple(all_in_names),
            out_names=tuple(out_names),
            lowering_input_output_aliases=(),
            sim_require_finite=True,
            sim_require_nnan=True,
            nc=nc,
        )
        return tuple(outs)

    devices = jax.devices()[:N_CORES]
    mesh = Mesh(np.asarray(devices), ("core",))
    in_specs = (PartitionSpec("core"),) * (n_params + n_outs)
    out_specs = (PartitionSpec("core"),) * n_outs
    sharded = jax.jit(
        shard_map(_body, mesh=mesh, in_specs=in_specs, out_specs=out_specs,
                  check_rep=False),
        donate_argnums=donate, keep_unused=True)

    _CACHE[("nc", rep)] = nc
    _CACHE[("exec", rep)] = (sharded, in_names, out_names, out_avals,
                             zero_shapes)
    return _CACHE[("exec", rep)]


def _run_cores(in_maps):
    """Run the SPMD program; in_maps is a list of 8 dicts name->np.ndarray."""
    sharded, in_names, out_names, out_avals, zero_shapes = _get_exec()
    concat_in = [
        np.concatenate([np.asarray(in_maps[c][n]) for c in range(N_CORES)],
                       axis=0)
        for n in in_names
    ]
    concat_zeros = [
        np.zeros((N_CORES * s[0], *s[1:]), dt) for (s, dt) in zero_shapes
    ]
    out_arrs = sharded(*concat_in, *concat_zeros)
    return [
        {n: np.asarray(out_arrs[i]).reshape(N_CORES, *out_avals[i].shape)[c]
         for i, n in enumerate(out_names)}
        for c in range(N_CORES)
    ]


def benchmark(in_maps, iters=10, rep=1):
    """Time repeated executions with device-resident inputs. Returns list of
    per-call wall seconds (axon RPC overhead included)."""
    import time
    import jax
    import jax.numpy as jnp
    from jax.sharding import Mesh, PartitionSpec, NamedSharding

    sharded, in_names, out_names, out_avals, zero_shapes = _get_exec(rep)
    devices = jax.devices()[:N_CORES]
    mesh = Mesh(np.asarray(devices), ("core",))
    shard = NamedSharding(mesh, PartitionSpec("core"))
    dev_in = [
        jax.device_put(
            np.concatenate([np.asarray(in_maps[c][n]) for c in range(N_CORES)],
                           axis=0), shard)
        for n in in_names
    ]
    jax.block_until_ready(dev_in)

    def make_zeros():
        zs = [jnp.zeros((N_CORES * s[0], *s[1:]), dt, device=shard)
              for (s, dt) in zero_shapes]
        jax.block_until_ready(zs)
        return zs

    out = sharded(*dev_in, *make_zeros())
    jax.block_until_ready(out)
    times = []
    for _ in range(iters):
        zs = make_zeros()
        t0 = time.perf_counter()
        out = sharded(*dev_in, *zs)
        jax.block_until_ready(out)
        times.append(time.perf_counter() - t0)
    return times


def _host_prep(hidden_states, W_qkv, b_qkv, W_dense, b_dense):
    _ZBIAS[0] = (not np.any(np.asarray(b_qkv))
                 and not np.any(np.asarray(b_dense)))
    hid = np.ascontiguousarray(
        np.asarray(hidden_states, dtype=np.float32).reshape(SEQ, HIDDEN))
    hid_t = np.ascontiguousarray(hid.T.astype(np.float16))  # [HIDDEN, SEQ]
    W_qkv = np.asarray(W_qkv, dtype=np.float32)
    b_qkv = np.asarray(b_qkv, dtype=np.float32)
    W_dense = np.asarray(W_dense, dtype=np.float32)
    b_dense = np.asarray(b_dense, dtype=np.float32)

    # rotary tables, computed in float32 exactly as the reference does
    inv_freq = (1.0 / (np.float32(10000.0) **
                       (np.arange(0, ROT, 2, dtype=np.float32)
                        / np.float32(ROT))))
    t = np.arange(SEQ, dtype=np.float32)
    freqs = t[:, None] * inv_freq[None, :]          # [SEQ, 16]
    cosf = np.cos(freqs).T                          # [16, SEQ]
    sinf = np.sin(freqs).T
    # row hl*32 + d: cos(emb[d mod 16]); sin carries the rotate_half sign
    cos_blk = np.concatenate([cosf, cosf], axis=0)      # [32, SEQ]
    sin_blk = np.concatenate([-sinf, sinf], axis=0)
    cos_t = np.tile(cos_blk, (HPC, 1)).astype(np.float16)  # [128, SEQ]
    sin_t = np.tile(sin_blk, (HPC, 1)).astype(np.float16)

    # additive causal masks for the 4 diagonal j-tiles of each i-block
    pj = np.arange(128)[:, None]
    fi = np.arange(SB)[None, :]
    mask = np.concatenate(
        [np.where(128 * t_ + pj <= fi, 0.0, NEG) for t_ in range(4)],
        axis=1).astype(__import__('ml_dtypes').bfloat16)  # [128, 4*SB]

    in_maps = []
    for c in range(N_CORES):
        heads = [HPC * c + i for i in range(HPC)]
        qcol = lambda h, d: h * 3 * HD + d
        kcol = lambda h, d: h * 3 * HD + HD + d
        vcol = lambda h, d: h * 3 * HD + 2 * HD + d
        perm = []
        perm += [qcol(h, d) for h in heads for d in range(ROT)]
        perm += [kcol(h, d) for h in heads for d in range(ROT)]
        perm += [qcol(h, d) for h in heads for d in range(ROT, HD)]
        perm += [kcol(h, d) for h in heads for d in range(ROT, HD)]
        perm = np.asarray(perm)
        vperm = np.asarray([vcol(h, d) for h in heads for d in range(HD)])

        w_qk = W_qkv[:, perm].astype(np.float16)     # [4096, 1024]
        w_qk = np.ascontiguousarray(
            w_qk.reshape(KT, 128, 8, 128).transpose(2, 0, 1, 3))
        w_v = np.ascontiguousarray(
            W_qkv[:, vperm].astype(np.float16).reshape(KT, 128, CW))
        w_d = np.ascontiguousarray(
            W_dense[:, c * CW:(c + 1) * CW].astype(np.float16).reshape(
                KT, 128, CW))
        in_maps.append({
            "hid_t": hid_t,
            "w_qk": w_qk,
            "w_v": w_v,
            "w_d": w_d,
            "b_qk": np.ascontiguousarray(b_qkv[perm].reshape(8, 128).T),
            "b_v": b_qkv[vperm].astype(np.float16).reshape(1, CW),
            "b_d": (b_dense[c * CW:(c + 1) * CW].astype(np.float16)
                    .reshape(1, CW)),
            "cos_in": cos_t,
            "sin_in": sin_t,
            "mask_in": mask,
            "ones_col_in": np.ones((128, 1), np.float32),
            "ones_row_in": np.ones((1, 128), np.float32),
            "ones_row16_in": np.ones((1, 128), np.float16),
        })
    return in_maps


def kernel(hidden_states, attention_mask=None, W_qkv=None, b_qkv=None,
           W_dense=None, b_dense=None, **_unused):
    in_maps = _host_prep(hidden_states, W_qkv, b_qkv, W_dense, b_dense)
    results = _run_cores(in_maps)
    full = np.concatenate([results[c]["out"] for c in range(N_CORES)], axis=1)
    return full.reshape(SEQ, 1, HIDDEN).astype(np.float32)


if __name__ == "__main__":
    rng = np.random.default_rng(0)
    ins = {
        "hidden_states": rng.standard_normal((SEQ, 1, HIDDEN),
                                             dtype=np.float32),
        "attention_mask": np.triu(np.ones((SEQ, SEQ), dtype=bool),
                                  1)[None, None],
        "W_qkv": (rng.standard_normal((HIDDEN, 3 * HIDDEN), dtype=np.float32)
                  * 0.02),
        "b_qkv": np.zeros(3 * HIDDEN, np.float32),
        "W_dense": (rng.standard_normal((HIDDEN, HIDDEN), dtype=np.float32)
                    * 0.02),
        "b_dense": np.zeros(HIDDEN, np.float32),
    }
    o = kernel(**ins)
    print("kernel output:", o.shape, o.dtype, float(np.abs(o).max()))

